# revision 8
# baseline (speedup 1.0000x reference)
"""DistSageConv Trainium2 kernel v4: SAGE mean-aggregation GNN on 8 NeuronCores.

Sharding: output nodes (dst) are split across the 8 cores (12500 each); feat is
replicated (bf16, padded to 256B rows) in every core's HBM. Per core:

  1. dma_gather (4 SWDGE queues round-robin) fetches feat[src] for its edges;
     int16 gather indices force 4 source banks of 25000 rows. Edges are laid
     out on the host into per-(phase, bank) runs sorted by dst-window, padded
     only at run tails to 128 (the max count over the 8 cores, so one program
     serves all cores).
  2. The PE segment-sums each 128-edge chunk into 128-dst windows via one-hot
     masks (bf16) generated on the DVE from a per-edge dst-in-window table
     compared against an iota row: psumT_w[f, d] += msgs_chunk.T @ mask.
     Chunks straddling window boundaries issue one matmul per window present
     (union over cores), with sentinel entries zeroing foreign edges.
  3. h_neigh normalization (x 1/deg) happens in the PSUM->SBUF copy; the
     output tile is featT_aug.T @ [W_self.T; b] + h_neighT.T @ W_neigh.T
     accumulated in one PSUM group, copied out on the scalar engine.
"""

import numpy as np
from dataclasses import dataclass

try:
    import ml_dtypes
    BF16 = ml_dtypes.bfloat16
except ImportError:  # pragma: no cover
    BF16 = None


@dataclass(frozen=True)
class Cfg:
    N: int = 100000          # nodes
    D: int = 64              # feature dim (in == out)
    C: int = 8               # cores
    WIN: int = 128           # dst window (one-hot width)
    NBANK: int = 4           # src banks (int16 gather index limit)
    PHASE_WINS: tuple = (7,) * 14   # windows per phase (sum == NW)
    MB: int = 32             # mask-gen batch (mm-slots per DVE op)

    @property
    def NPC(self):
        return self.N // self.C

    @property
    def NW(self):
        return -(-self.NPC // self.WIN)

    @property
    def BROWS(self):
        return self.N // self.NBANK


CFG = Cfg()
SENTINEL = 1000.0  # dst-in-window for padding/foreign edges; never matches iota


@dataclass
class Structure:
    TC: int                  # total arena chunk slots
    E_struct: int            # padded edge count (TC*128)
    TM: int                  # total mm-slots
    mm_slot: np.ndarray      # [TM] arena slot of each mm
    mm_win: np.ndarray       # [TM] global window of each mm
    phase_chunk0: list       # per phase: first arena slot
    phase_chunk1: list
    phase_mm0: list          # per phase: first mm-slot
    phase_mm1: list
    phase_w0: list
    phase_nw: list
    win_mm0: np.ndarray      # [NW] first mm-slot of window
    win_mm1: np.ndarray      # [NW] last mm-slot (inclusive)
    calls: list              # per phase: list of (bank, slot0, nslots, idxcol0)


@dataclass
class CoreData:
    idx_blob: np.ndarray     # [128, E_struct//16] int16
    dstloc: np.ndarray       # [128, TM] bf16
    invdegb: np.ndarray      # [128, NW*WIN] f32
    featT_aug: np.ndarray    # [D+1, NW*WIN] f32


def preprocess(feat, src, dst, W_self, W_neigh, b, cfg: Cfg = CFG):
    N, D, C = cfg.N, cfg.D, cfg.C
    NPC, WIN, NW = cfg.NPC, cfg.WIN, cfg.NW
    NBANK, BROWS = cfg.NBANK, cfg.BROWS
    phases = list(cfg.PHASE_WINS)
    NP = len(phases)
    assert sum(phases) == NW

    feat = np.asarray(feat, dtype=np.float32)
    src32 = np.asarray(src).astype(np.int64)
    dst32 = np.asarray(dst).astype(np.int64)
    E = src32.shape[0]
    W_self = np.asarray(W_self, dtype=np.float32)
    W_neigh = np.asarray(W_neigh, dtype=np.float32)
    b = np.asarray(b, dtype=np.float32)

    deg = np.bincount(dst32, minlength=N).astype(np.float32)
    invdeg = (1.0 / np.maximum(deg, 1.0)).astype(np.float32)

    core = dst32 // NPC
    dloc = dst32 - core * NPC
    wl = dloc // WIN
    dwin = (dloc - wl * WIN).astype(np.float32)
    bank = src32 // BROWS
    lidx = (src32 - bank * BROWS).astype(np.int16)

    w0_of_p = np.concatenate([[0], np.cumsum(phases)])[:-1].astype(np.int64)
    ph_of_w = np.repeat(np.arange(NP), phases)

    # per-(core, w, b) counts -> per-(core, p, b) runs and window extents
    cnt_kwb = np.bincount(
        (core * NW + wl) * NBANK + bank, minlength=C * NW * NBANK
    ).reshape(C, NW, NBANK)

    # run lengths and chunk counts per (p, b)
    run_len = np.zeros((C, NP, NBANK), np.int64)
    for p in range(NP):
        ws = slice(w0_of_p[p], w0_of_p[p] + phases[p])
        run_len[:, p, :] = cnt_kwb[:, ws, :].sum(axis=1)
    chunks_pb = -(-run_len.max(axis=0) // 128)       # [NP, NBANK]
    TC = int(chunks_pb.sum())
    E_struct = TC * 128

    # arena slot base per (p, b) in (p, b) order
    slot_base = np.zeros((NP, NBANK), np.int64)
    acc = 0
    for p in range(NP):
        for bb in range(NBANK):
            slot_base[p, bb] = acc
            acc += chunks_pb[p, bb]
    phase_chunk0 = [int(slot_base[p, 0]) for p in range(NP)]
    phase_chunk1 = phase_chunk0[1:] + [TC]

    # per-edge padded position: order (core, p, b, w, arrival)
    p_e = ph_of_w[wl]
    gid_e = ((core * NP + p_e) * NBANK + bank) * NW + wl
    order = np.argsort(gid_e, kind="stable")
    ks = gid_e[order]
    starts = np.concatenate([[0], np.flatnonzero(ks[1:] != ks[:-1]) + 1])
    runlen = np.diff(np.concatenate([starts, [E]]))
    rank_in_wgroup = np.empty(E, np.int64)
    rank_in_wgroup[order] = np.arange(E) - np.repeat(starts, runlen)
    # offset of window w within core k's (p, b) run
    woff = np.zeros((C, NW, NBANK), np.int64)
    for p in range(NP):
        w0p, nwp = w0_of_p[p], phases[p]
        cum = np.cumsum(cnt_kwb[:, w0p:w0p + nwp, :], axis=1)
        woff[:, w0p + 1:w0p + nwp, :] = cum[:, :-1, :]
    runpos = woff[core, wl, bank] + rank_in_wgroup
    pos = slot_base[p_e, bank] * 128 + runpos
    assert (runpos < run_len[core, p_e, bank]).all()

    # mm-slots ordered (p, w, b, chunk): union over cores of windows present
    mm_slot, mm_win = [], []
    phase_mm0, phase_mm1 = [], []
    win_mm0 = np.full(NW, -1, np.int64)
    win_mm1 = np.full(NW, -1, np.int64)
    for p in range(NP):
        phase_mm0.append(len(mm_slot))
        for w in range(w0_of_p[p], w0_of_p[p] + phases[p]):
            for bb in range(NBANK):
                lo_k = woff[:, w, bb]                       # [C]
                hi_k = lo_k + cnt_kwb[:, w, bb]
                c_lo = int((lo_k // 128).min())
                c_hi = int((-(-hi_k // 128)).max())
                for c in range(c_lo, c_hi):
                    if (np.minimum(hi_k, (c + 1) * 128)
                            > np.maximum(lo_k, c * 128)).any():
                        if win_mm0[w] < 0:
                            win_mm0[w] = len(mm_slot)
                        win_mm1[w] = len(mm_slot)
                        mm_slot.append(int(slot_base[p, bb] + c))
                        mm_win.append(w)
        phase_mm1.append(len(mm_slot))
    mm_slot = np.array(mm_slot)
    mm_win = np.array(mm_win)
    TM = len(mm_slot)
    assert (win_mm0 >= 0).all()

    # gather calls per (p, b)
    calls = [[] for _ in range(NP)]
    idxcol = 0
    for p in range(NP):
        for bb in range(NBANK):
            ns = int(chunks_pb[p, bb])
            if ns == 0:
                continue
            h = ns // 2
            for (o, n2) in (((0, h)) , ((h, ns - h))):
                if n2 == 0:
                    continue
                calls[p].append((bb, int(slot_base[p, bb]) + o, n2, idxcol))
                idxcol += n2 * 8
    assert idxcol == E_struct // 16

    st = Structure(
        TC=TC, E_struct=E_struct, TM=TM, mm_slot=mm_slot, mm_win=mm_win,
        phase_chunk0=phase_chunk0, phase_chunk1=phase_chunk1,
        phase_mm0=phase_mm0, phase_mm1=phase_mm1,
        phase_w0=[int(x) for x in w0_of_p], phase_nw=phases,
        win_mm0=win_mm0, win_mm1=win_mm1, calls=calls,
    )

    # per-core arrays
    lidx_pad = np.zeros((C, E_struct), np.int16)
    wl_pad = np.full((C, E_struct), -1, np.int64)
    dwin_pad = np.zeros((C, E_struct), np.float32)
    lidx_pad[core, pos] = lidx
    wl_pad[core, pos] = wl
    dwin_pad[core, pos] = dwin

    # dstloc per mm-slot: [C, 128, TM]
    lane_w = wl_pad.reshape(C, TC, 128)       # window of each lane, -1 pad
    lane_d = dwin_pad.reshape(C, TC, 128)
    sl = lane_w[:, mm_slot, :]                # [C, TM, 128]
    sd = lane_d[:, mm_slot, :]
    dst_mm = np.where(sl == mm_win[None, :, None], sd, SENTINEL)
    dstloc_all = dst_mm.transpose(0, 2, 1).astype(BF16)   # [C, 128, TM]

    cores = []
    for k in range(C):
        blocks = []
        for p in range(NP):
            for (bb, s0, nslots, _c0) in calls[p]:
                seg = lidx_pad[k, s0 * 128:(s0 + nslots) * 128]
                blocks.append(seg.reshape(-1, 16).T)
        idx_blob = np.tile(np.concatenate(blocks, axis=1), (8, 1))

        v = np.ones(NW * WIN, np.float32)
        v[:NPC] = invdeg[k * NPC:(k + 1) * NPC]
        invdegb = np.tile(v, (128, 1)).astype(BF16)

        ft = np.zeros((D + 1, NW * WIN), BF16)
        ft[:D, :NPC] = feat[k * NPC:(k + 1) * NPC].T.astype(BF16)
        ft[D, :] = 1.0
        cores.append(CoreData(idx_blob=idx_blob, dstloc=dstloc_all[k],
                              invdegb=invdegb, featT_aug=ft))

    featb = np.zeros((N, 128), dtype=BF16)
    featb[:, :D] = feat.astype(BF16)
    consts = {
        "WselfT_aug": np.concatenate([W_self.T, b[None, :]], 0).astype(BF16),
        "WneighT": np.ascontiguousarray(np.tile(W_neigh.T, (2, 1)).astype(BF16)),
        "iota": np.tile(np.arange(128, dtype=BF16), (128, 1)),
        "featb": featb,
    }
    return st, cores, consts


def build_program(st: Structure, cfg: Cfg = CFG):
    import concourse.bacc as bacc
    import concourse.mybir as mybir
    import concourse.tile as tile

    D, WIN, NW = cfg.D, cfg.WIN, cfg.NW
    NP = len(cfg.PHASE_WINS)
    f32 = mybir.dt.float32
    bf16 = mybir.dt.bfloat16
    AL = mybir.AluOpType

    nc = bacc.Bacc("TRN2", target_bir_lowering=False, debug=False,
                   num_swdge_queues=4)

    featb_d = nc.dram_tensor("featb", [cfg.N, 128], bf16, kind="ExternalInput")
    featT_d = nc.dram_tensor("featT", [D + 1, NW * WIN], bf16, kind="ExternalInput")
    idx_d = nc.dram_tensor("idx", [128, st.E_struct // 16], mybir.dt.int16,
                           kind="ExternalInput")
    dstloc_d = nc.dram_tensor("dstloc", [128, st.TM], bf16, kind="ExternalInput")
    invdegb_d = nc.dram_tensor("invdegb", [128, NW * WIN], bf16, kind="ExternalInput")
    wself_d = nc.dram_tensor("WselfT_aug", [D + 1, D], bf16, kind="ExternalInput")
    wneigh_d = nc.dram_tensor("WneighT", [2 * D, D], bf16, kind="ExternalInput")
    iota_d = nc.dram_tensor("iota", [128, 128], bf16, kind="ExternalInput")
    out_d = nc.dram_tensor("out", [NW * WIN, D], f32, kind="ExternalOutput")

    with tile.TileContext(nc) as tc:
        with (
            tc.tile_pool(name="const", bufs=1) as cpool,
            tc.tile_pool(name="arena", bufs=4) as apool,
            tc.tile_pool(name="mask", bufs=3) as mpool,
            tc.tile_pool(name="featT", bufs=2) as fpool,
            tc.tile_pool(name="aggT", bufs=6) as gpool,
            tc.tile_pool(name="invd", bufs=2) as ipool,
            tc.tile_pool(name="outb", bufs=2) as opool,
            tc.tile_pool(name="psA", bufs=4, space="PSUM") as psA,
            tc.tile_pool(name="psO", bufs=2, space="PSUM") as psO,
        ):
            idx_sb = cpool.tile([128, st.E_struct // 16], mybir.dt.int16,
                                tag="idx")
            nc.sync.dma_start(idx_sb[:], idx_d[:, :])
            dstloc_sb = cpool.tile([128, st.TM], bf16, tag="dstloc")
            nc.sync.dma_start(dstloc_sb[:], dstloc_d[:, :])
            iota_sb = cpool.tile([128, 128], bf16, tag="iota")
            nc.sync.dma_start(iota_sb[:], iota_d[:, :])
            wself_sb = cpool.tile([D + 1, D], bf16, tag="wself")
            nc.sync.dma_start(wself_sb[:], wself_d[:, :])
            wneigh_sb = cpool.tile([2 * D, D], bf16, tag="wneigh")
            nc.sync.dma_start(wneigh_sb[:], wneigh_d[:, :])
            featT_all = cpool.tile([D + 1, NW * WIN], bf16, tag="featTa")
            nc.sync.dma_start(featT_all[:], featT_d[:, :])

            qn = 0
            for p in range(NP):
                pc0, pc1 = st.phase_chunk0[p], st.phase_chunk1[p]
                pm0, pm1 = st.phase_mm0[p], st.phase_mm1[p]
                w0, nwv = st.phase_w0[p], st.phase_nw[p]
                n0 = w0 * WIN

                arena = apool.tile([128, pc1 - pc0, 128], bf16, tag="arena")
                for (bb, s0, nslots, c0) in st.calls[p]:
                    nc.gpsimd.dma_gather(
                        arena[:, s0 - pc0:s0 - pc0 + nslots, :],
                        featb_d[bb * cfg.BROWS:(bb + 1) * cfg.BROWS, :],
                        idx_sb[:, c0:c0 + nslots * 8],
                        nslots * 128,
                        nslots * 128,
                        128,
                        single_packet=False,
                        queue_num=qn % 4,
                    )
                    qn += 1

                invd_t = ipool.tile([128, nwv * WIN], bf16, tag="invd")
                nc.sync.dma_start(invd_t[:], invdegb_d[:, n0:n0 + nwv * WIN])

                # masks for this phase's mm-slots, in MB-sized batches
                masks = {}
                for m0 in range(pm0, pm1, cfg.MB):
                    mb = min(cfg.MB, pm1 - m0)
                    mt = mpool.tile([128, mb, 128], bf16, tag="mask")
                    nc.vector.tensor_tensor(
                        mt[:],
                        iota_sb[:].unsqueeze(1).to_broadcast([128, mb, 128]),
                        dstloc_sb[:, m0:m0 + mb].unsqueeze(2)
                        .to_broadcast([128, mb, 128]),
                        AL.is_equal,
                    )
                    masks[m0] = mt

                outb = opool.tile([128, nwv, D], f32, tag="outb")
                for wl0 in range(0, nwv, 2):
                    pair = [wl0] + ([wl0 + 1] if wl0 + 1 < nwv else [])
                    ps_half = [psA.tile([128, WIN], f32, tag="psA",
                                        name=f"psh{p}_{wl0}_{hh}")
                               for hh in range(len(pair))]
                    aggT = gpool.tile([128, WIN], bf16, tag="aggT")
                    # interleave the two windows' matmuls on separate
                    # column groups of the PE array
                    mms = []
                    for hi, wli in enumerate(pair):
                        w = w0 + wli
                        mma, mmz = int(st.win_mm0[w]), int(st.win_mm1[w])
                        mms.append([(m, hi, m == mma, m == mmz)
                                    for m in range(mma, mmz + 1)])
                    order = []
                    for t in range(max(len(x) for x in mms)):
                        for x in mms:
                            if t < len(x):
                                order.append(x[t])
                    for (m, hi, st_, sp_) in order:
                        mt = masks[pm0 + ((m - pm0) // cfg.MB) * cfg.MB]
                        j = (m - pm0) % cfg.MB
                        nc.tensor.matmul(
                            ps_half[hi][hi * D:(hi + 1) * D, :],
                            lhsT=arena[:, st.mm_slot[m] - pc0, 0:D],
                            rhs=mt[:, j, :],
                            start=st_, stop=sp_,
                            tile_position=(0, hi * D),
                        )
                    for hi, wli in enumerate(pair):
                        nc.vector.scalar_tensor_tensor(
                            aggT[hi * D:(hi + 1) * D, :],
                            ps_half[hi][hi * D:(hi + 1) * D, :], 1.0,
                            invd_t[hi * D:(hi + 1) * D,
                                   wli * WIN:(wli + 1) * WIN],
                            AL.mult, AL.mult,
                        )
                    for hi, wli in enumerate(pair):
                        psum_o = psO.tile([128, D], f32, tag="psO")
                        nc.tensor.matmul(
                            psum_o[:],
                            lhsT=featT_all[:, n0 + wli * WIN:
                                            n0 + (wli + 1) * WIN],
                            rhs=wself_sb[:],
                            start=True, stop=False,
                        )
                        nc.tensor.matmul(
                            psum_o[:],
                            lhsT=aggT[hi * D:(hi + 1) * D, :],
                            rhs=wneigh_sb[hi * D:(hi + 1) * D, :],
                            start=False, stop=True,
                        )
                        nc.scalar.copy(outb[:, wli, :], psum_o[:])
                nc.sync.dma_start(
                    out_d[n0:n0 + nwv * WIN, :]
                    .rearrange("(w p) d -> p w d", p=128),
                    outb[:],
                )

    nc.compile()
    return nc


def run(feat, src, dst, W_self, W_neigh, b, cfg: Cfg = CFG, trace=False):
    from concourse.bass_utils import run_bass_kernel_spmd

    st, cores, consts = preprocess(feat, src, dst, W_self, W_neigh, b, cfg)
    nc = build_program(st, cfg)

    in_maps = []
    for k in range(cfg.C):
        cd = cores[k]
        in_maps.append({
            "featb": consts["featb"],
            "featT": cd.featT_aug,
            "idx": cd.idx_blob,
            "dstloc": cd.dstloc,
            "invdegb": cd.invdegb,
            "WselfT_aug": consts["WselfT_aug"],
            "WneighT": consts["WneighT"],
            "iota": consts["iota"],
        })
    res = run_bass_kernel_spmd(nc, in_maps, core_ids=list(range(cfg.C)),
                               trace=trace)
    out = np.empty((cfg.N, cfg.D), np.float32)
    for k in range(cfg.C):
        out[k * cfg.NPC:(k + 1) * cfg.NPC] = res.results[k]["out"][:cfg.NPC]
    return out, res


def kernel(feat, src, dst, W_self, W_neigh, b):
    out, _ = run(feat, src, dst, W_self, W_neigh, b)
    return out
